# revision 10
# baseline (speedup 1.0000x reference)
"""Multi-head attention (b=8, n=1024, dim=1024, 16 heads x 64) on 8 TRN2 cores.

Sharding: data-parallel over batch (one batch element per core), SPMD NEFF.

Per-core dataflow (all matmuls f32r = full-speed PE with ~1e-4 matmul error):
  A) xT = PE-transpose(x);  v[t,f] = xT.T @ w_v (staged to DRAM with a ones
     column per head);  qkT[f,t] = w_qk.T @ xT in head-pair order
  B) per head: S^T[j,i] = kT.T @ qT (K=64);  P^T = exp(scale*S^T) on ACT;
     [outT|l] = [v|1].T @ P^T (M=65, softmax denominator for free);
     normalize via reciprocal + K=1 ones-broadcast matmul
  C) out = cat.T @ w_out + b_out (natural layout), DMA out

Scheduling structure:
  - one PSUM pool for the whole kernel (tags io/mm/s = 2+2+4 banks)
  - phase-B tiles (pt/v_ext/rec) allocated BELOW phase A's region so B's head 0
    can overlap A's tail; cat + w_out live in the region A frees
  - all DMA on the HWDGE ring (measured faster than splitting rings)
"""
import sys

sys.path.insert(0, "/opt/trn_rl_repo")

import numpy as np

import concourse.mybir as mybir
import concourse.tile as tile
from concourse import bacc
from concourse.bass_utils import run_bass_kernel_spmd
from concourse.masks import make_identity

FP32 = mybir.dt.float32
F32R = mybir.dt.float32r
AF = mybir.ActivationFunctionType
MUL = mybir.AluOpType.mult
ADD = mybir.AluOpType.add

N = 1024      # seq len
D = 1024      # model dim
H = 16        # heads
HD = 64       # head dim
SCALE = HD ** -0.5
NC_ = 8       # n cores = batch
KC = D // 128  # 8 contraction chunks


def build_attention_nc(repeats=1):
    nc = bacc.Bacc("TRN2", target_bir_lowering=False, debug=False, num_devices=1)

    x = nc.dram_tensor("x", [N, D], FP32, kind="ExternalInput").ap()
    w_qkv = nc.dram_tensor("w_qkv", [D, 3 * D], FP32, kind="ExternalInput").ap()
    w_out = nc.dram_tensor("w_out", [D, D], FP32, kind="ExternalInput").ap()
    b_out = nc.dram_tensor("b_out", [1, D], FP32, kind="ExternalInput").ap()
    out = nc.dram_tensor("out", [N, D], FP32, kind="ExternalOutput").ap()

    with tile.TileContext(nc) as tc:
        for _rep in range(repeats):
            _body(nc, tc, x, w_qkv, w_out, b_out, out)

    nc.compile()
    return nc


def _body(nc, tc, x, w_qkv, w_out, b_out, out):
        with (
            tc.tile_pool(name="persist", bufs=1) as pp,
            tc.tile_pool(name="dram", bufs=1, space="DRAM") as dp,
            tc.tile_pool(name="ps", bufs=2, space="PSUM") as ps,
            tc.tile_pool(name="pbp", bufs=4) as pbp,
            tc.tile_pool(name="pb", bufs=3) as pb,
            tc.tile_pool(name="pb2", bufs=2) as pb2,
        ):
            # v (natural layout) staged to DRAM with a ones column per head:
            # [j, h*65+d], col 64 of each head block == 1.0
            v_dram = dp.tile([N, H * 65], F32R)

            ident = pp.tile([128, 128], FP32, tag="ident")
            make_identity(nc, ident[:])
            ones = pp.tile([1, 128], FP32, tag="ones")
            nc.vector.memset(ones[:], 1.0)
            # persistent reciprocal-row tiles for the paired 1/l broadcast:
            # row 0 -> even head, row 32 -> odd head (32-aligned partitions);
            # rows 1-31 stay 1.0 and are zeroed out by sel
            rec_t = [pp.tile([33, 512], FP32, tag=f"rec{i}", name=f"rec{i}") for i in range(2)]
            nc.vector.memset(rec_t[0][:], 1.0)
            nc.vector.memset(rec_t[1][:], 1.0)
            # head-pair selector: row 0 -> cols 0-63 (even head), row 32 ->
            # cols 64-127 (odd head)
            sel = pp.tile([33, 128], FP32, tag="sel")
            nc.vector.memset(sel[:], 0.0)
            nc.vector.memset(sel[0:1, 0:64], 1.0)
            nc.vector.memset(sel[32:33, 64:128], 1.0)

            qkT = [pp.tile([128, N], F32R, tag=f"qk{f}", name=f"qk{f}") for f in range(H)]

            # ---------------- Phase A: xT, v, qkT ----------------
            with (
                tc.tile_pool(name="pa", bufs=1) as pa,
                tc.tile_pool(name="pa2", bufs=3) as pa2,
            ):
                xT = [pa.tile([128, N], F32R, tag=f"xt{c}", name=f"xt{c}") for c in range(KC)]
                stage = [pa.tile([128, H * 65], FP32, tag=f"stg{i}", name=f"stg{i}") for i in range(2)]
                nc.vector.memset(stage[0][:], 1.0)
                nc.vector.memset(stage[1][:], 1.0)

                # first x chunks on HWDGE; weights go on the SWDGE ring
                x_tiles = [pa2.tile([128, D], FP32, tag="x", name=f"x{i}") for i in range(2)]
                for tc_i in range(2):
                    nc.sync.dma_start(x_tiles[tc_i][:], x[tc_i * 128:(tc_i + 1) * 128, :])

                wv = [pa.tile([128, KC, 512], F32R, tag=f"wv{fs}", name=f"wv{fs}") for fs in range(2)]
                for fs in range(2):
                    nc.sync.dma_start(
                        wv[fs][:],
                        w_qkv[:, 2 * D + fs * 512:2 * D + (fs + 1) * 512]
                        .rearrange("(ko p) f -> p ko f", p=128)
                        .bitcast(F32R),
                    )

                # warm the ACT exp table before phase B needs it
                warm = pa.tile([1, 2], FP32, tag="warm")
                nc.scalar.activation(warm[:], ident[0:1, 0:2], AF.Exp)

                # transpose chunk tc, then immediately compute v rows for
                # those tokens (only needs this chunk's transposes + wv)
                for tc_i in range(8):
                    if tc_i >= 2:
                        x_sb = pa2.tile([128, D], FP32, tag="x", name=f"x{tc_i}")
                        nc.sync.dma_start(x_sb[:], x[tc_i * 128:(tc_i + 1) * 128, :])
                        x_tiles.append(x_sb)
                    x_sb = x_tiles[tc_i]
                    for dc in range(8):
                        tr_ps = ps.tile([128, 512], FP32, tag="io", name="tr")
                        nc.tensor.transpose(
                            tr_ps[:, 0:128], x_sb[:, dc * 128:(dc + 1) * 128], ident[:]
                        )
                        nc.vector.tensor_copy(
                            xT[dc][:, tc_i * 128:(tc_i + 1) * 128], tr_ps[:, 0:128]
                        )
                    stg = stage[tc_i % 2]
                    for fs in range(2):
                        mm = ps.tile([128, 512], FP32, tag="mm")
                        for kc in range(KC):
                            nc.tensor.matmul(
                                mm[:],
                                xT[kc][:, tc_i * 128:(tc_i + 1) * 128],
                                wv[fs][:, kc, :],
                                start=(kc == 0),
                                stop=(kc == KC - 1),
                            )
                        nc.vector.tensor_copy(
                            stg.rearrange("p (h e) -> p h e", e=65)[:, fs * 8:(fs + 1) * 8, 0:64],
                            mm[:].rearrange("p (h e) -> p h e", e=64),
                        )
                    nc.sync.dma_start(
                        v_dram[tc_i * 128:(tc_i + 1) * 128, :], stg[:].bitcast(F32R)
                    )

                # Software pipeline: emit qk projection for head-pair f,
                # then the attention blocks for pair f-1. The exp stream on
                # ACT overlaps the qk matmuls on PE. Normalized attention
                # output is written back into the (dead) q rows of qkT, so
                # cat[c] IS qkT[c] (disjoint partition halves per head).
                def emit_qk(f):
                    for fc in (f, 8 + f):
                        wc = pa2.tile([128, KC, 128], F32R, tag="wc")
                        nc.sync.dma_start(
                            wc[:],
                            w_qkv[:, fc * 128:(fc + 1) * 128]
                            .rearrange("(ko p) f -> p ko f", p=128)
                            .bitcast(F32R),
                        )
                        for ic in range(2):
                            mm = ps.tile([128, 512], FP32, tag="mm")
                            for kc in range(KC):
                                nc.tensor.matmul(
                                    mm[:],
                                    wc[:, kc, :],
                                    xT[kc][:, ic * 512:(ic + 1) * 512],
                                    start=(kc == 0),
                                    stop=(kc == KC - 1),
                                )
                            nc.vector.tensor_copy(qkT[fc][:, ic * 512:(ic + 1) * 512], mm[:])

                def emit_attention_pair(fp):
                    h_e, h_o = 2 * fp, 2 * fp + 1
                    qc = fp
                    qt, kt = qkT[qc], qkT[8 + qc]
                    vxs = []
                    for h in (h_e, h_o):
                        v_ext = pb.tile([128, KC, 65], F32R, tag="vx")
                        nc.sync.dma_start(
                            v_ext[:],
                            v_dram[:, h * 65:(h + 1) * 65]
                            .rearrange("(ko p) d -> p ko d", p=128),
                        )
                        vxs.append(v_ext)
                    for ic in range(2):
                        o_e = ps.tile([128, 512], FP32, tag="io", name="oe")
                        o_o = ps.tile([128, 512], FP32, tag="io", name="oo")
                        o_of = {h_e: o_e, h_o: o_o}
                        pts = []
                        # S matmuls for the two heads sit in disjoint PE row
                        # groups (partitions 0-63 / 64-127) -> HW-concurrent
                        for jc in range(KC):
                            s_ps = ps.tile([128, 2, 512], FP32, tag="s")
                            for hi, po in ((0, 0), (1, 64)):
                                nc.tensor.matmul(
                                    s_ps[:, hi, :],
                                    kt[po:po + 64, jc * 128:(jc + 1) * 128],
                                    qt[po:po + 64, ic * 512:(ic + 1) * 512],
                                    start=True,
                                    stop=True,
                                )
                            pt = pbp.tile([128, 2, 512], F32R, tag="pt")
                            nc.scalar.activation(
                                pt[:].rearrange("p a b -> p (a b)"),
                                s_ps[:].rearrange("p a b -> p (a b)"),
                                AF.Exp,
                                scale=SCALE,
                            )
                            pts.append(pt)
                            if jc >= 1:
                                for hi, h in ((0, h_e), (1, h_o)):
                                    nc.tensor.matmul(
                                        o_of[h][0:65, :],
                                        vxs[hi][:, jc - 1, :],
                                        pts[jc - 1][:, hi, :],
                                        start=(jc - 1 == 0),
                                        stop=False,
                                    )
                        for hi, h in ((0, h_e), (1, h_o)):
                            nc.tensor.matmul(
                                o_of[h][0:65, :],
                                vxs[hi][:, KC - 1, :],
                                pts[KC - 1][:, hi, :],
                                start=False,
                                stop=True,
                            )
                        rec2 = rec_t[(2 * fp + ic) % 2]
                        nc.vector.reciprocal(rec2[0:1, :], o_e[64:65, :])
                        nc.vector.reciprocal(rec2[32:33, :], o_o[64:65, :])
                        b_ps = ps.tile([128, 512], FP32, tag="mm", name="bps")
                        nc.tensor.matmul(
                            b_ps[:], sel[:], rec2[:], start=True, stop=True
                        )
                        bc_sb = pb2.tile([128, 512], FP32, tag="bc")
                        nc.vector.tensor_copy(bc_sb[:], b_ps[:])
                        for hi, (h, po) in enumerate(((h_e, 0), (h_o, 64))):
                            nc.vector.tensor_tensor(
                                qkT[qc][po:po + 64, ic * 512:(ic + 1) * 512],
                                o_of[h][0:64, :],
                                bc_sb[po:po + 64, :],
                                MUL,
                            )

                for f in range(KC + 1):
                    if f < KC:
                        emit_qk(f)
                    if f >= 1:
                        emit_attention_pair(f - 1)

            # ---------------- Phase C: out = cat.T @ w_out + b_out ----------
            cat = qkT  # normalized attention output lives in the q/k tiles
            with tc.tile_pool(name="pbc", bufs=1) as pbc:
                w_out_sb = [pbc.tile([128, D], F32R, tag=f"wo{c}", name=f"wo{c}") for c in range(KC)]
                for kc in range(KC):
                    nc.sync.dma_start(
                        w_out_sb[kc][:],
                        w_out[kc * 128:(kc + 1) * 128, :].bitcast(F32R),
                    )
                with tc.tile_pool(name="pc", bufs=3) as pc:
                    b_row = pc.tile([1, D], FP32, tag="brow")
                    nc.sync.dma_start(b_row[:], b_out[:])
                    b_sb = pc.tile([128, D], FP32, tag="bsb")
                    for half in range(2):
                        bb_ps = ps.tile([128, 512], FP32, tag="io", name="bb")
                        nc.tensor.matmul(
                            bb_ps[:],
                            ones[:],
                            b_row[:, half * 512:(half + 1) * 512],
                            start=True,
                            stop=True,
                        )
                        nc.vector.tensor_copy(
                            b_sb[:, half * 512:(half + 1) * 512], bb_ps[:]
                        )
                    for tc_i in range(8):
                        out_sb = pc.tile([128, D], FP32, tag="osb")
                        for mc in range(2):
                            c_ps = ps.tile([128, 512], FP32, tag="mm", name="cps")
                            for kc in range(KC):
                                nc.tensor.matmul(
                                    c_ps[:],
                                    cat[kc][:, tc_i * 128:(tc_i + 1) * 128],
                                    w_out_sb[kc][:, mc * 512:(mc + 1) * 512],
                                    start=(kc == 0),
                                    stop=(kc == KC - 1),
                                )
                            nc.vector.tensor_tensor(
                                out_sb[:, mc * 512:(mc + 1) * 512],
                                c_ps[:],
                                b_sb[:, mc * 512:(mc + 1) * 512],
                                ADD,
                            )
                        nc.sync.dma_start(
                            out[tc_i * 128:(tc_i + 1) * 128, :], out_sb[:]
                        )


_NC_CACHE = None


def _get_nc():
    global _NC_CACHE
    if _NC_CACHE is None:
        _NC_CACHE = build_attention_nc()
    return _NC_CACHE


def kernel(x, w_qkv, w_out, b_out, _trace=False, **_kw):
    x = np.ascontiguousarray(x, dtype=np.float32)
    w_qkv = np.ascontiguousarray(w_qkv, dtype=np.float32)
    w_out = np.ascontiguousarray(w_out, dtype=np.float32)
    b_row = np.ascontiguousarray(b_out, dtype=np.float32).reshape(1, D)

    nc = _get_nc()
    in_maps = [
        {"x": x[b], "w_qkv": w_qkv, "w_out": w_out, "b_out": b_row}
        for b in range(NC_)
    ]
    res = run_bass_kernel_spmd(nc, in_maps, core_ids=list(range(NC_)), trace=_trace)
    out = np.stack([res.results[b]["out"] for b in range(NC_)], axis=0)
    if _trace:
        return out, res
    return out

